# revision 6
# baseline (speedup 1.0000x reference)
# Trainium2 Bass kernel for nn_BertAdapter_SLT_49933289783411
#
# Reference computation:
#   y   = tt_linear(x) + bias          (TT-factorized 768->768 linear)
#   out = x + gelu_exact(y)
#
# Key math: the TT cores with ranks [1,5,5,5,5,5,1] factor the 768x768
# weight as W = A @ B with A:(768,5), B:(5,768).  We precompute A,B on
# host (tiny, exact) and run a rank-5 bottleneck matmul on device.
#
# Sharding: data-parallel over the batch dim (8 batch elements -> 8 cores).
# Each core handles x_c:(512,768).  Host pre-transposes x_c to x^T (f-major)
# so the contraction dim lands on SBUF partitions; the device computes
#   t3    = A^T @ x^T              (5,512)   PSUM accumulate over 6 f-chunks
#   y^T_j = B_j^T @ t3             (128,512) per 128-feature output chunk j
#   o^T_j = x^T_j + gelu(y^T_j + bias_j)
# and the host transposes the gathered o^T back.

import numpy as np

import concourse.bass as bass
import concourse.bacc as bacc
import concourse.mybir as mybir
import concourse.tile as tile
from concourse.bass_utils import run_bass_kernel_spmd

HID = 768
ROWS = 512          # rows per core (one batch element)
NCORES = 8
FCH = 6             # 768 / 128 feature chunks
RANK = 5
F32 = mybir.dt.float32

_CACHE = {}


def _build_program(act=None):
    if act is None:
        act = mybir.ActivationFunctionType.Gelu
    nc = bacc.Bacc(None, target_bir_lowering=False)
    xt = nc.dram_tensor("xt", [128, FCH * ROWS], F32, kind="ExternalInput")
    a_p = nc.dram_tensor("a_p", [128, FCH * RANK], F32, kind="ExternalInput")
    bm_p = nc.dram_tensor("bm_p", [RANK, HID], F32, kind="ExternalInput")
    bias_p = nc.dram_tensor("bias_p", [128, FCH], F32, kind="ExternalInput")
    outt = nc.dram_tensor("outt", [128, FCH * ROWS], F32, kind="ExternalOutput")

    with tile.TileContext(nc) as tc:
        with (
            tc.tile_pool(name="const", bufs=1) as cpool,
            tc.tile_pool(name="xs", bufs=1) as xpool,
            tc.tile_pool(name="work", bufs=3) as wpool,
            tc.tile_pool(name="ps_t3", bufs=1, space="PSUM") as tpool,
            tc.tile_pool(name="ps_o", bufs=3, space="PSUM") as opool,
        ):
            a_sb = cpool.tile([128, FCH * RANK], F32)
            nc.sync.dma_start(a_sb[:], a_p[:])
            bm_sb = cpool.tile([RANK, HID], F32)
            nc.sync.dma_start(bm_sb[:], bm_p[:])
            bias_sb = cpool.tile([128, FCH], F32)
            nc.sync.dma_start(bias_sb[:], bias_p[:])

            x_sb = xpool.tile([128, FCH * ROWS], F32)
            for c in range(FCH):
                nc.sync.dma_start(
                    x_sb[:, c * ROWS : (c + 1) * ROWS],
                    xt[:, c * ROWS : (c + 1) * ROWS],
                )

            # t3 = A^T @ x^T, accumulated over the 6 feature chunks
            t3_ps = tpool.tile([RANK, ROWS], F32)
            for c in range(FCH):
                nc.tensor.matmul(
                    t3_ps[:],
                    a_sb[:, c * RANK : (c + 1) * RANK],
                    x_sb[:, c * ROWS : (c + 1) * ROWS],
                    start=(c == 0),
                    stop=(c == FCH - 1),
                )
            t3_sb = cpool.tile([RANK, ROWS], F32)
            nc.vector.tensor_copy(t3_sb[:], t3_ps[:])

            # per output chunk: y^T_j = B_j^T @ t3 ; out = x^T_j + gelu(y + bias)
            for j in range(FCH):
                o_ps = opool.tile([128, ROWS], F32)
                nc.tensor.matmul(
                    o_ps[:],
                    bm_sb[:, j * 128 : (j + 1) * 128],
                    t3_sb[:],
                    start=True,
                    stop=True,
                )
                g_sb = wpool.tile([128, ROWS], F32)
                nc.scalar.activation(
                    g_sb[:],
                    o_ps[:],
                    act,
                    bias=bias_sb[:, j : j + 1],
                    scale=1.0,
                )
                o_sb = wpool.tile([128, ROWS], F32)
                nc.vector.tensor_add(
                    o_sb[:], g_sb[:], x_sb[:, j * ROWS : (j + 1) * ROWS]
                )
                nc.sync.dma_start(outt[:, j * ROWS : (j + 1) * ROWS], o_sb[:])

    nc.finalize()
    return nc


def _get_program():
    if "nc" not in _CACHE:
        _CACHE["nc"] = _build_program()
    return _CACHE["nc"]


def _host_prep(hidden_states, bias, cores):
    """Collapse TT cores to rank-5 factors and pack to device layouts."""
    c0, c1, c2, c3, c4, c5 = [c.astype(np.float64) for c in cores]
    A = np.einsum("iv,vjw,wkx->ijkx", c0[0], c1, c2).reshape(HID, RANK)
    Bm = np.einsum("xpy,yqz,zr->xpqr", c3, c4, c5[:, :, 0]).reshape(RANK, HID)

    a_p = np.ascontiguousarray(
        A.reshape(FCH, 128, RANK).transpose(1, 0, 2).reshape(128, FCH * RANK)
    ).astype(np.float32)
    bm_p = np.ascontiguousarray(Bm).astype(np.float32)
    bias_p = np.ascontiguousarray(
        bias.astype(np.float32).reshape(FCH, 128).T
    )

    # x^T packed: xt[p, c*ROWS + m] = x[m, c*128 + p]
    xts = []
    for c in range(NCORES):
        xc = hidden_states[c]  # (512, 768)
        xt = np.ascontiguousarray(
            xc.T.reshape(FCH, 128, ROWS).transpose(1, 0, 2).reshape(128, FCH * ROWS)
        ).astype(np.float32)
        xts.append(xt)
    return a_p, bm_p, bias_p, xts


def _unpack_out(outt_list):
    """outt[p, j*ROWS + m] = out[m, j*128 + p] -> (8, 512, 768)."""
    outs = []
    for outt in outt_list:
        o = outt.reshape(128, FCH, ROWS).transpose(2, 1, 0).reshape(ROWS, HID)
        outs.append(o)
    return np.stack(outs, axis=0).astype(np.float32)


def run(inputs, trace=False, **spmd_kwargs):
    hidden_states = np.asarray(inputs["hidden_states"], dtype=np.float32)
    bias = np.asarray(inputs["bias"], dtype=np.float32)
    cores = [np.asarray(inputs[f"core{i}"], dtype=np.float32) for i in range(6)]

    a_p, bm_p, bias_p, xts = _host_prep(hidden_states, bias, cores)
    nc = _get_program()
    in_maps = [
        {"xt": xts[c], "a_p": a_p, "bm_p": bm_p, "bias_p": bias_p}
        for c in range(NCORES)
    ]
    res = run_bass_kernel_spmd(
        nc, in_maps, core_ids=list(range(NCORES)), trace=trace, **spmd_kwargs
    )
    out = _unpack_out([res.results[c]["outt"] for c in range(NCORES)])
    if trace:
        return out, res
    return out


def kernel(**inputs):
    return run(inputs)


# revision 7
# speedup vs baseline: 1.1256x; 1.1256x over previous
# Trainium2 Bass kernel for nn_BertAdapter_SLT_49933289783411
#
# Reference computation:
#   y   = tt_linear(x) + bias          (TT-factorized 768->768 linear)
#   out = x + gelu_exact(y)
#
# Key math: the TT cores with ranks [1,5,5,5,5,5,1] factor the 768x768
# weight as W = A @ B with A:(768,5), B:(5,768).  We precompute A,B on
# host (tiny, exact) and run a rank-5 bottleneck matmul on device.
#
# Sharding: data-parallel over the batch dim (8 batch elements -> 8 cores).
# Each core handles x_c:(512,768).  Host pre-transposes x_c to x^T (f-major)
# so the contraction dim lands on SBUF partitions; the device computes
#   t3    = A^T @ x^T              (5,512)   PSUM accumulate over 6 f-chunks
#   y^T_j = B_j^T @ t3             (128,512) per 128-feature output chunk j
#   o^T_j = x^T_j + gelu(y^T_j + bias_j)
# and the host transposes the gathered o^T back.
#
# The matmul path runs in bf16 (x cast on device, A/B cast on host): the
# TT branch contributes only ~4% of output magnitude, so bf16 there costs
# ~2e-4 relative output error while halving PE passes (fp32 matmul = 2
# passes on TRN2).  The residual add stays fully fp32.

import numpy as np
import ml_dtypes

import concourse.bass as bass
import concourse.bacc as bacc
import concourse.mybir as mybir
import concourse.tile as tile
from concourse.bass_utils import run_bass_kernel_spmd

HID = 768
ROWS = 512          # rows per core (one batch element)
NCORES = 8
FCH = 6             # 768 / 128 feature chunks
RANK = 5
F32 = mybir.dt.float32
BF16 = mybir.dt.bfloat16

N_WARMUP = 20       # dummy PE matmuls to trip the HAM clock un-throttle

_CACHE = {}


def _build_program(act=None):
    if act is None:
        act = mybir.ActivationFunctionType.Gelu
    nc = bacc.Bacc(None, target_bir_lowering=False)
    xt = nc.dram_tensor("xt", [128, FCH * ROWS], F32, kind="ExternalInput")
    a_p = nc.dram_tensor("a_p", [128, FCH * RANK], BF16, kind="ExternalInput")
    bm_p = nc.dram_tensor("bm_p", [RANK, HID], BF16, kind="ExternalInput")
    bias_p = nc.dram_tensor("bias_p", [128, FCH], F32, kind="ExternalInput")
    outt = nc.dram_tensor("outt", [128, FCH * ROWS], F32, kind="ExternalOutput")

    with tile.TileContext(nc) as tc:
        with (
            tc.tile_pool(name="const", bufs=1) as cpool,
            tc.tile_pool(name="xs", bufs=1) as xpool,
            tc.tile_pool(name="work", bufs=3) as wpool,
            tc.tile_pool(name="ps_t3", bufs=1, space="PSUM") as tpool,
            tc.tile_pool(name="ps_o", bufs=3, space="PSUM") as opool,
            tc.tile_pool(name="ps_w", bufs=1, space="PSUM") as wps_pool,
        ):
            # --- PE warmup: garbage matmuls so the HAM clock gate opens
            # (PE defaults to 1.2 GHz; ~3.4us of activity unlocks 2.4 GHz)
            wsb = cpool.tile([128, 128], BF16)
            nc.gpsimd.memset(wsb[:], 0.0)
            wps = wps_pool.tile([128, 128], F32)
            for _ in range(N_WARMUP):
                nc.tensor.matmul(wps[:], wsb[:], wsb[:], start=True, stop=True)

            # --- constants on the SWDGE queue (keeps the Sync HWDGE queue
            # free for the big x transfers)
            a_sb = cpool.tile([128, FCH * RANK], BF16)
            nc.gpsimd.dma_start(a_sb[:], a_p[:])
            bm_sb = cpool.tile([RANK, HID], BF16)
            nc.gpsimd.dma_start(bm_sb[:], bm_p[:])
            bias_sb = cpool.tile([128, FCH], F32)
            nc.gpsimd.dma_start(bias_sb[:], bias_p[:])

            # --- x loads interleaved with bf16 cast + mm1 accumulation
            x_sb = xpool.tile([128, FCH * ROWS], F32)
            xb_sb = xpool.tile([128, FCH * ROWS], BF16)
            t3_ps = tpool.tile([RANK, ROWS], F32)
            for c in range(FCH):
                sl = slice(c * ROWS, (c + 1) * ROWS)
                nc.sync.dma_start(x_sb[:, sl], xt[:, sl])
                nc.vector.tensor_copy(xb_sb[:, sl], x_sb[:, sl])
                nc.tensor.matmul(
                    t3_ps[:],
                    a_sb[:, c * RANK : (c + 1) * RANK],
                    xb_sb[:, sl],
                    start=(c == 0),
                    stop=(c == FCH - 1),
                )

            # t3 -> SBUF as bf16 (ScalarE is idle here; keep DVE free)
            t3_sb = cpool.tile([RANK, ROWS], BF16)
            nc.scalar.copy(t3_sb[:], t3_ps[:])

            # --- per output chunk: y^T_j = B_j^T @ t3 ; o = x + gelu(y + b)
            for j in range(FCH):
                sl = slice(j * ROWS, (j + 1) * ROWS)
                o_ps = opool.tile([128, ROWS], F32)
                nc.tensor.matmul(
                    o_ps[:],
                    bm_sb[:, j * 128 : (j + 1) * 128],
                    t3_sb[:],
                    start=True,
                    stop=True,
                )
                g_sb = wpool.tile([128, ROWS], F32)
                nc.scalar.activation(
                    g_sb[:],
                    o_ps[:],
                    act,
                    bias=bias_sb[:, j : j + 1],
                    scale=1.0,
                )
                o_sb = wpool.tile([128, ROWS], F32)
                nc.vector.tensor_add(
                    o_sb[:], g_sb[:], x_sb[:, sl]
                )
                nc.sync.dma_start(outt[:, sl], o_sb[:])

    nc.finalize()
    return nc


def _get_program():
    if "nc" not in _CACHE:
        _CACHE["nc"] = _build_program()
    return _CACHE["nc"]


def _host_prep(hidden_states, bias, cores):
    """Collapse TT cores to rank-5 factors and pack to device layouts."""
    c0, c1, c2, c3, c4, c5 = [c.astype(np.float64) for c in cores]
    A = np.einsum("iv,vjw,wkx->ijkx", c0[0], c1, c2).reshape(HID, RANK)
    Bm = np.einsum("xpy,yqz,zr->xpqr", c3, c4, c5[:, :, 0]).reshape(RANK, HID)

    a_p = np.ascontiguousarray(
        A.reshape(FCH, 128, RANK).transpose(1, 0, 2).reshape(128, FCH * RANK)
    ).astype(ml_dtypes.bfloat16)
    bm_p = np.ascontiguousarray(Bm).astype(ml_dtypes.bfloat16)
    bias_p = np.ascontiguousarray(
        bias.astype(np.float32).reshape(FCH, 128).T
    )

    # x^T packed: xt[p, c*ROWS + m] = x[m, c*128 + p]
    xts = []
    for c in range(NCORES):
        xc = hidden_states[c]  # (512, 768)
        xt = np.ascontiguousarray(
            xc.T.reshape(FCH, 128, ROWS).transpose(1, 0, 2).reshape(128, FCH * ROWS)
        ).astype(np.float32)
        xts.append(xt)
    return a_p, bm_p, bias_p, xts


def _unpack_out(outt_list):
    """outt[p, j*ROWS + m] = out[m, j*128 + p] -> (8, 512, 768)."""
    outs = []
    for outt in outt_list:
        o = outt.reshape(128, FCH, ROWS).transpose(2, 1, 0).reshape(ROWS, HID)
        outs.append(o)
    return np.stack(outs, axis=0).astype(np.float32)


def run(inputs, trace=False, **spmd_kwargs):
    hidden_states = np.asarray(inputs["hidden_states"], dtype=np.float32)
    bias = np.asarray(inputs["bias"], dtype=np.float32)
    cores = [np.asarray(inputs[f"core{i}"], dtype=np.float32) for i in range(6)]

    a_p, bm_p, bias_p, xts = _host_prep(hidden_states, bias, cores)
    nc = _get_program()
    in_maps = [
        {"xt": xts[c], "a_p": a_p, "bm_p": bm_p, "bias_p": bias_p}
        for c in range(NCORES)
    ]
    res = run_bass_kernel_spmd(
        nc, in_maps, core_ids=list(range(NCORES)), trace=trace, **spmd_kwargs
    )
    out = _unpack_out([res.results[c]["outt"] for c in range(NCORES)])
    if trace:
        return out, res
    return out


def kernel(**inputs):
    return run(inputs)
